# revision 2
# baseline (speedup 1.0000x reference)
"""Paged attention (decode, GQA, RoPE) Trainium2 Bass kernel.

Problem: B=16 seqs, H=32 q heads, KVH=8 kv heads, D=128, T=4096 max ctx,
S=65536 cache slots.  out[b,h,:] = softmax(rope_q(q) @ rope_k(gather(K))^T
* 1/sqrt(D), masked to ctx_len) @ gather(V).

Sharding: one KV head per NeuronCore (8 cores).  Every core runs the SAME
program over all 16 sequences -> perfectly balanced; per-core inputs are the
head's K/V cache slice (interleaved K|V per slot for 1KB gather rows), the 4
query heads of its group, slot-offset tables and RoPE tables.

Math trick: with c_t[d]=cos(t*w_{d%64}), s_t[d]=sin(t*w_{d%64}),
  rope_q(q) . rope_k(K_t) = qr . (K_t*c_t) + v . (K_t*s_t)
where qr = rope(q) and v = -rotate_half(qr).  So K-side RoPE becomes two
elementwise multiplies of the transposed K tile with host-precomputed
cos/sin tables, folded into two accumulating matmuls.
"""

import math
import numpy as np
from contextlib import ExitStack

import concourse.bass as bass
import concourse.tile as tile
from concourse import bacc, mybir
from concourse.bass_utils import run_bass_kernel_spmd
from concourse.masks import make_identity

F32 = mybir.dt.float32
I32 = mybir.dt.int32

D = 128        # head dim
G = 4          # query heads per kv head (GQA group)
P = 128        # partitions / tile token count
ROPE_BASE = 10000.0
NEG_BIG = -30.0  # masked-score fill; exp(-30) ~ 9e-14 is negligible vs O(1) weights


def _plan(context_lens, n_chunk):
    """Static schedule: seq-major list of (seq, ktile, is_first, is_last, L)."""
    sched = []
    for b, ctx in enumerate(context_lens):
        nt = max(1, math.ceil(int(ctx) / P))
        for k in range(nt):
            L = min(P, int(ctx) - k * P)
            sched.append((b, k, k == 0, k == nt - 1, L))
    nt_total = len(sched)
    n_chunks = math.ceil(nt_total / n_chunk)
    return sched, nt_total, n_chunks


def build_kernel(context_lens, T, S, n_chunk=16):
    """Build the per-core Bass program (identical across cores)."""
    B = len(context_lens)
    sched, NT, NCHUNK = _plan(context_lens, n_chunk)
    NTP = NCHUNK * n_chunk

    nc = bacc.Bacc("TRN2", target_bir_lowering=False, debug=False, num_devices=8)

    kv = nc.dram_tensor("kv", [S, 2 * D], F32, kind="ExternalInput")
    offs = nc.dram_tensor("offs", [P, NTP], I32, kind="ExternalInput")
    qrT = nc.dram_tensor("qrT", [D, B * G], F32, kind="ExternalInput")
    vT = nc.dram_tensor("vT", [D, B * G], F32, kind="ExternalInput")
    cosT = nc.dram_tensor("cosT", [D, T], F32, kind="ExternalInput")
    sinT = nc.dram_tensor("sinT", [D, T], F32, kind="ExternalInput")
    out = nc.dram_tensor("out", [B, G, D], F32, kind="ExternalOutput")

    with tile.TileContext(nc) as tc, ExitStack() as ctx:
        singles = ctx.enter_context(tc.tile_pool(name="singles", bufs=1))
        chunks = ctx.enter_context(tc.tile_pool(name="chunks", bufs=2))
        kts = ctx.enter_context(tc.tile_pool(name="kts", bufs=3))
        small = ctx.enter_context(tc.tile_pool(name="small", bufs=3))
        seqbufs = ctx.enter_context(tc.tile_pool(name="seqbufs", bufs=2))
        pkt = ctx.enter_context(tc.tile_pool(name="pkt", bufs=2, space="PSUM"))
        psc = ctx.enter_context(tc.tile_pool(name="psc", bufs=2, space="PSUM"))
        pat = ctx.enter_context(tc.tile_pool(name="pat", bufs=2, space="PSUM"))
        pout = ctx.enter_context(tc.tile_pool(name="pout", bufs=2, space="PSUM"))

        cos_sb = singles.tile([D, T], F32)
        nc.sync.dma_start(cos_sb[:], cosT.ap())
        sin_sb = singles.tile([D, T], F32)
        nc.sync.dma_start(sin_sb[:], sinT.ap())
        qrT_sb = singles.tile([D, B * G], F32)
        nc.sync.dma_start(qrT_sb[:], qrT.ap())
        vT_sb = singles.tile([D, B * G], F32)
        nc.sync.dma_start(vT_sb[:], vT.ap())
        offs_sb = singles.tile([P, NTP], I32)
        nc.sync.dma_start(offs_sb[:], offs.ap())
        id128 = singles.tile([P, P], F32)
        make_identity(nc, id128[:])
        idG = singles.tile([G, G], F32)
        make_identity(nc, idG[:])

        # per-seq state, allocated when the seq's first tile appears
        out_ps = [None] * B
        sums_sb = [None] * B
        max_tiles_per_seq = max(1, math.ceil(T / P))

        for ci in range(NCHUNK):
            chunk = chunks.tile([P, n_chunk * 2 * D], F32, tag="chunk")
            for j in range(n_chunk):
                jj = ci * n_chunk + j
                if jj >= NT:
                    break
                # HW indirect DMA consumes ONE index per output partition row:
                # offsets [128,1] -> 128 rows of 2D contiguous elements each.
                nc.gpsimd.indirect_dma_start(
                    out=chunk[:, j * 2 * D:(j + 1) * 2 * D],
                    out_offset=None,
                    in_=kv.ap(),
                    in_offset=bass.IndirectOffsetOnAxis(
                        ap=offs_sb[:, jj:jj + 1], axis=0
                    ),
                )
            for j in range(n_chunk):
                jj = ci * n_chunk + j
                if jj >= NT:
                    break
                b, k, first, last, L = sched[jj]
                t0 = k * P
                kview = chunk[:, j * 2 * D: j * 2 * D + D]
                vview = chunk[:, j * 2 * D + D: (j + 1) * 2 * D]

                if first:
                    out_ps[b] = pout.tile([G, D], F32, tag="out_ps", name=f"out_ps_{b}")
                    sums_sb[b] = seqbufs.tile(
                        [G, max_tiles_per_seq], F32, tag="sums", name=f"sums_{b}"
                    )

                # K^T via PE transpose
                kt_ps = pkt.tile([P, P], F32, tag="kt_ps")
                nc.tensor.transpose(out=kt_ps[:], in_=kview, identity=id128[:])

                # RoPE via cos/sin tables (PSUM -> SBUF elementwise muls)
                ktc = kts.tile([P, P], F32, tag="ktc")
                nc.vector.tensor_mul(ktc[:], kt_ps[:], cos_sb[:, t0:t0 + P])
                kts_t = kts.tile([P, P], F32, tag="kts_t")
                nc.vector.tensor_mul(kts_t[:], kt_ps[:], sin_sb[:, t0:t0 + P])

                # scores[g, t] = qr_b . K^T c + v_b . K^T s   (accumulate in PSUM)
                sc_ps = psc.tile([G, P], F32, tag="sc_ps")
                nc.tensor.matmul(
                    sc_ps[:], lhsT=qrT_sb[:, b * G:(b + 1) * G], rhs=ktc[:],
                    start=True, stop=False,
                )
                nc.tensor.matmul(
                    sc_ps[:], lhsT=vT_sb[:, b * G:(b + 1) * G], rhs=kts_t[:],
                    start=False, stop=True,
                )
                if L < P:
                    nc.vector.memset(sc_ps[:, L:P], NEG_BIG)

                # attn = exp(scores); row-sum into the seq's sums column
                attn = small.tile([G, P], F32, tag="attn")
                nc.scalar.activation(
                    out=attn[:], in_=sc_ps[:],
                    func=mybir.ActivationFunctionType.Exp,
                    accum_out=sums_sb[b][:, k:k + 1],
                )

                # attn^T for the AV matmul
                at_ps = pat.tile([P, G], F32, tag="at_ps")
                nc.tensor.transpose(out=at_ps[:], in_=attn[:], identity=idG[:])
                at_sb = small.tile([P, G], F32, tag="at_sb")
                nc.vector.tensor_copy(at_sb[:], at_ps[:])

                # out_b += attn^T.T @ V
                nc.tensor.matmul(
                    out_ps[b][:], lhsT=at_sb[:], rhs=vview,
                    start=first, stop=last,
                )

                if last:
                    nt_b = k + 1
                    stot = small.tile([G, 1], F32, tag="stot")
                    nc.vector.tensor_reduce(
                        out=stot[:], in_=sums_sb[b][:, :nt_b],
                        axis=mybir.AxisListType.X, op=mybir.AluOpType.add,
                    )
                    rec = small.tile([G, 1], F32, tag="rec")
                    nc.vector.reciprocal(rec[:], stot[:])
                    ob = small.tile([G, D], F32, tag="ob")
                    nc.vector.tensor_scalar_mul(ob[:], out_ps[b][:], rec[:])
                    nc.sync.dma_start(out.ap()[b], ob[:])

    nc.compile()
    return nc


def _host_prep(query, k_cache, v_cache, slot_tables, context_lens, n_chunk=16):
    """Build per-core input maps (numpy only)."""
    B, H, d = query.shape
    S, KVH, _ = k_cache.shape
    T = slot_tables.shape[1]
    assert d == D
    n_cores = KVH
    ctx = np.asarray(context_lens).astype(np.int64)

    sched, NT, NCHUNK = _plan(ctx, n_chunk)
    NTP = NCHUNK * n_chunk

    st = np.asarray(slot_tables).astype(np.int32)  # [B, T]
    offs = np.zeros((P, NTP), np.int32)
    for jj, (b, k, _, _, _) in enumerate(sched):
        offs[:, jj] = st[b, k * P:k * P + P]

    # RoPE tables: c[d, t] = cos(t * w_{d%64})
    inv_freq = 1.0 / (ROPE_BASE ** (np.arange(0, D, 2, dtype=np.float32) / D))  # [64]
    tpos = np.arange(T, dtype=np.float32)
    ang = tpos[None, :] * inv_freq[:, None]            # [64, T]
    cosT = np.concatenate([np.cos(ang), np.cos(ang)], axis=0).astype(np.float32)
    sinT = np.concatenate([np.sin(ang), np.sin(ang)], axis=0).astype(np.float32)

    # query-side RoPE at position ctx-1, folded scale
    qpos = (ctx - 1).astype(np.float32)                # [B]
    angq = qpos[:, None] * inv_freq[None, :]           # [B, 64]
    cq = np.concatenate([np.cos(angq), np.cos(angq)], axis=1)[:, None, :]  # [B,1,D]
    sq = np.concatenate([np.sin(angq), np.sin(angq)], axis=1)[:, None, :]
    q = np.asarray(query, np.float32)

    def rot_half(x):
        return np.concatenate([-x[..., D // 2:], x[..., :D // 2]], axis=-1)

    qr = q * cq + rot_half(q) * sq                     # [B, H, D]
    scale = np.float32(1.0 / np.sqrt(D))
    qr_s = (qr * scale).astype(np.float32)
    v_s = (-rot_half(qr) * scale).astype(np.float32)

    kc = np.asarray(k_cache, np.float32)
    vc = np.asarray(v_cache, np.float32)

    in_maps = []
    for c in range(n_cores):
        kv_comb = np.concatenate([kc[:, c, :], vc[:, c, :]], axis=1)  # [S, 256]
        # [D, B*G] with column b*G+g = head (c*G+g) of seq b
        qrT = np.ascontiguousarray(
            qr_s[:, c * G:(c + 1) * G, :].transpose(2, 0, 1).reshape(D, B * G)
        )
        vT = np.ascontiguousarray(
            v_s[:, c * G:(c + 1) * G, :].transpose(2, 0, 1).reshape(D, B * G)
        )
        in_maps.append({
            "kv": np.ascontiguousarray(kv_comb),
            "offs": offs,
            "qrT": qrT,
            "vT": vT,
            "cosT": cosT,
            "sinT": sinT,
        })
    return in_maps, (B, H, KVH, T, S)


_CACHE = {}


def kernel(query, k_cache, v_cache, slot_tables, context_lens):
    query = np.asarray(query)
    out_dtype = query.dtype
    B, H, d = query.shape
    S, KVH, _ = np.asarray(k_cache).shape
    T = np.asarray(slot_tables).shape[1]
    n_chunk = 16

    in_maps, _ = _host_prep(query, k_cache, v_cache, slot_tables, context_lens,
                            n_chunk)

    ctx_key = tuple(int(x) for x in np.asarray(context_lens))
    key = (ctx_key, B, H, KVH, T, S)
    if key not in _CACHE:
        _CACHE[key] = build_kernel(np.asarray(context_lens), T, S, n_chunk)
    nc = _CACHE[key]

    res = run_bass_kernel_spmd(nc, in_maps, core_ids=list(range(KVH)))
    # core c holds query heads c*G..(c+1)*G-1
    out = np.concatenate([res.results[c]["out"] for c in range(KVH)], axis=1)
    return out.astype(out_dtype, copy=False)
